# revision 14
# baseline (speedup 1.0000x reference)
"""CrossAttention (B=1, S=4096, H=8, DH=40) on 8 Trainium2 NeuronCores.

Sharding: tensor-parallel over the 8 heads — core h computes head h's full
attention plus its partial output projection; the host sums the 8 partials
and adds the bias.

v3 dataflow (vs v2: AV lagged a full chunk, HAM-warm junk matmuls, balanced
chunk-0, host-prepacked weight images, fast reciprocal, lean tail):
  preamble  wA/wB/wv/woT as single prepacked DMAs; xT in 1024-col slices
            across 3 issuing queues; dummy ACT preloads the exp table;
            qkA drains on ScalarE (idle until the ACT stream starts)
  qT/kT     packed [104, 4096] images (q@0/k@64 and k@0/q@64) so the two
            ST matmuls of a j-group run on disjoint PE row groups
  ST        [128j, 512i] pairs -> exp on ScalarE [128, 1024] (the wall:
            ~16.7M exps at 1 elem/cycle/lane = ~142us)
  AV        lag-16: av(c) consumes chunk c-1's exp tiles, so steady slots
            are uniform 2 ST + 2 AV and never wait on a fresh ACT; chunk 7
            carries av(6) and av(7) (lag-1) together
  warm      junk matmuls fill the ~260ns/slot PE idle of a ScalarE-paced
            steady state -- without them the HAM clock gate re-throttles
            the PE to 1.2 GHz (measured: half the kernel ran cold)
  norm      r rows 40/104 -> SBUF DMA to partition 0 -> recip_approx_fast
            -> K=1 ones-matmul broadcast -> mult into oT_lo/oT_hi; runs at
            the last slot of even chunks, right after the pair's AV stop
  Y         [128, 320] = oT.T @ Wo_h.T, spread 1 tile/slot over odd chunks
"""

import os

import ml_dtypes
import numpy as np

import concourse.bass as bass
import concourse.mybir as mybir
from concourse import bass_utils, masks
from concourse.tile import TileContext

S = 4096
D = 320
H = 8
DH = 40
N_CORES = 8
CHUNK = 512               # i-chunk width (one fp32 PSUM bank)
VW = 41                   # v' stationary width: 40 v cols, ones col 40
GJ = 2                    # j-tiles per exp group (2 PSUM banks)
SCALE = float(DH) ** -0.5
QKW = 104
JW = 384                  # junk keep-warm matmul moving width

F32 = mybir.dt.float32
BF16 = mybir.dt.bfloat16
EXP = mybir.ActivationFunctionType.Exp
LOG = mybir.ActivationFunctionType.Ln
MULT = mybir.AluOpType.mult

KCH = (128, 128, 64)      # K chunks of D=320

_COMPILED = {}


def _split_sync_waits(nc, max_waits=1):
    """This walrus build rejects instructions with more than one sync wait.
    Spill the excess onto same-engine nops placed just before the
    instruction (engine streams execute in program order, so all waits are
    satisfied before the instruction issues)."""
    for f in nc.m.functions:
        for bb in f.blocks:
            out = []
            changed = False
            for inst in bb.instructions:
                si = inst.sync_info
                if si is not None and si.on_wait and len(si.on_wait) > max_waits:
                    waits = list(si.on_wait)
                    for i in range(max_waits, len(waits), max_waits):
                        nop = mybir.InstNoOp(
                            name=nc.get_next_instruction_name(),
                            engine=inst.engine,
                            bass_nofuse=True,
                            sync_info=mybir.SyncInfo(
                                on_wait=waits[i:i + max_waits], on_update=[]),
                        )
                        out.append(nop)
                    inst.sync_info = mybir.SyncInfo(
                        on_wait=waits[:max_waits],
                        on_update=list(si.on_update or []))
                    changed = True
                out.append(inst)
            if changed:
                bb.instructions = out


def _build(s=None, split=True):
    s = s or S
    n_chunks = s // CHUNK
    jt = s // 128
    gpc = jt // GJ            # exp groups per chunk
    tpc = CHUNK // 128        # s-tiles per chunk
    nc = bass.Bass('TRN2', target_bir_lowering=False, debug=False)

    xT_d = nc.dram_tensor('xT', [D, s], BF16, kind='ExternalInput').ap()
    wA_d = nc.dram_tensor('wA', [128, 3 * QKW], BF16, kind='ExternalInput').ap()
    wB_d = nc.dram_tensor('wB', [128, 3 * QKW], BF16, kind='ExternalInput').ap()
    wv_d = nc.dram_tensor('wv', [128, 3 * DH], BF16, kind='ExternalInput').ap()
    woT_d = nc.dram_tensor('woT', [QKW, D], BF16, kind='ExternalInput').ap()
    out_d = nc.dram_tensor('out', [s, D], F32, kind='ExternalOutput').ap()

    with TileContext(nc) as tc:
        with tc.tile_pool(name='const', bufs=1) as cpool, \
             tc.tile_pool(name='big', bufs=1) as big, \
             tc.tile_pool(name='pt', bufs=18) as ptp, \
             tc.tile_pool(name='work', bufs=3) as wkp, \
             tc.tile_pool(name='ps_st', bufs=2, space='PSUM') as ps_st, \
             tc.tile_pool(name='ps_small', bufs=2, space='PSUM') as ps_small, \
             tc.tile_pool(name='ps_av', bufs=1, space='PSUM') as ps_av, \
             tc.tile_pool(name='ps_warm', bufs=1, space='PSUM') as ps_warm:

            # ---- constants & inputs ----
            ident = cpool.tile([128, 128], F32, tag='ident')
            masks.make_identity(nc, ident[:, :])

            wA = cpool.tile([128, 3 * QKW], BF16, tag='wA')
            wB = cpool.tile([128, 3 * QKW], BF16, tag='wB')
            wv_sb = cpool.tile([128, 3 * DH], BF16, tag='wv')
            woT2 = cpool.tile([QKW, D], BF16, tag='woT')
            nc.sync.dma_start(wA[:, :], wA_d)
            nc.sync.dma_start(wB[:, :], wB_d)
            nc.scalar.dma_start(wv_sb[:, :], wv_d)
            nc.gpsimd.dma_start(woT2[:, :], woT_d)
            ones1 = cpool.tile([1, DH], F32, tag='ones1')
            nc.vector.memset(ones1[0:1, :], 1.0)

            # xT in 1024-col slices round-robined over 3 issuing queues so
            # the transfers overlap and proj(c) can start as slice c//2 lands
            xt0 = big.tile([128, s], BF16, tag='xt0')
            xt1 = big.tile([128, s], BF16, tag='xt1')
            xt2 = big.tile([64, s], BF16, tag='xt2')
            xts = (xt0, xt1, xt2)
            for sl in range(s // 1024):
                ss = slice(sl * 1024, (sl + 1) * 1024)
                nc.sync.dma_start(xt0[:, ss], xT_d[0:128, ss])
                nc.scalar.dma_start(xt1[:, ss], xT_d[128:256, ss])
                nc.gpsimd.dma_start(xt2[:, ss], xT_d[256:320, ss])

            qkA = big.tile([QKW, s], BF16, tag='qkA')
            qkB = big.tile([QKW, s], BF16, tag='qkB')
            vT = big.tile([VW, s], F32, tag='vT')
            vsb = big.tile([128, jt * VW], BF16, tag='vsb')
            oT_lo = big.tile([DH, s], BF16, tag='oT_lo')
            oT_hi = big.tile([QKW, s], BF16, tag='oT_hi')

            # base-32 memset (DVE base must be 32-aligned); proj then
            # overwrites rows 32..39 with v, leaving row 40 = 1.0
            nc.vector.memset(vT[32:VW, :], 1.0)

            # dummy ACT: forces the exp table-set load (~2.7us) to happen
            # during the DMA wait instead of delaying the first real exp
            sc = wkp.tile([1, 16], F32, tag='sc')
            nc.scalar.activation(sc[0:1, :], ident[0:1, 0:16], EXP)

            # ---- helpers ----
            def proj(dst, w_sb, c, ww, drain='vector'):
                ps = ps_small.tile([QKW, CHUNK], F32, tag='small')
                for ci, kk in enumerate(KCH):
                    nc.tensor.matmul(
                        ps[0:ww, :],
                        w_sb[0:kk, ci * ww:(ci + 1) * ww],
                        xts[ci][0:kk, c * CHUNK:(c + 1) * CHUNK],
                        start=(ci == 0), stop=(ci == 2))
                if drain == 'scalar':
                    nc.scalar.copy(dst[:, c * CHUNK:(c + 1) * CHUNK],
                                   ps[0:ww, :])
                else:
                    nc.vector.tensor_copy(dst[:, c * CHUNK:(c + 1) * CHUNK],
                                          ps[0:ww, :])

            def proj_qk(c):
                # q/k projections for chunk c into one 2-bank ps_st tile
                # (idle until the ST stream starts): B in the low bank, A in
                # the high bank. 2-chunk WAR distance keeps the PE free of
                # the drain-copy round-trip that stalled v3's chunk 0.
                ps = ps_st.tile([128, GJ * CHUNK], F32, tag='st2')
                cs = slice(c * CHUNK, (c + 1) * CHUNK)
                for ci, kk in enumerate(KCH):
                    nc.tensor.matmul(
                        ps[0:QKW, 0:CHUNK],
                        wB[0:kk, ci * QKW:(ci + 1) * QKW],
                        xts[ci][0:kk, cs],
                        start=(ci == 0), stop=(ci == 2))
                for ci, kk in enumerate(KCH):
                    nc.tensor.matmul(
                        ps[0:QKW, CHUNK:2 * CHUNK],
                        wA[0:kk, ci * QKW:(ci + 1) * QKW],
                        xts[ci][0:kk, cs],
                        start=(ci == 0), stop=(ci == 2))
                nc.vector.tensor_copy(qkB[:, cs], ps[0:QKW, 0:CHUNK])
                nc.scalar.copy(qkA[:, cs], ps[0:QKW, CHUNK:2 * CHUNK])

            def transpose_v(j):
                tp = ps_small.tile([128, VW], F32, tag='small')
                nc.tensor.transpose(tp[:, 0:VW], vT[:, j * 128:(j + 1) * 128],
                                    ident[0:VW, 0:VW])
                nc.vector.tensor_copy(vsb[:, j * VW:(j + 1) * VW],
                                      tp[:, 0:VW])

            def warm(n=1):
                # keep-warm matmuls: a ScalarE-paced steady state leaves the
                # PE idle ~260ns/slot, which trips the HAM activity monitor
                # and halves the PE clock. Junk matmuls fill the gap.
                w = ps_warm.tile([128, CHUNK], F32, tag='warm', name='warm')
                for _ in range(n):
                    nc.tensor.matmul(w[0:QKW, 0:JW], wA[:, 0:QKW],
                                     xt0[:, 0:JW], start=True, stop=True)

            av_box = [None]

            def av_mm(c_src, pt, g):
                # two j-tile matmuls of group g, consuming chunk c_src's exp
                # tiles: even chunk at PE cols 0-40, odd at 64-104. The last
                # pair accumulates in the ps_warm bank (free in chunk 7, no
                # junk there) so it never WARs on the previous pair's drain.
                lo = (c_src % 2 == 0)
                for k in range(GJ):
                    j = GJ * g + k
                    if j == 0 and lo:
                        pool = ps_warm if c_src >= 6 else ps_av
                        av_box[0] = pool.tile([128, CHUNK], F32,
                                              tag='warm' if c_src >= 6
                                              else 'av',
                                              name='av')
                    av = av_box[0]
                    if lo:
                        nc.tensor.matmul(
                            av[0:VW, :], vsb[:, j * VW:(j + 1) * VW],
                            pt[:, k * CHUNK:(k + 1) * CHUNK],
                            start=(j == 0), stop=(j == jt - 1))
                    else:
                        nc.tensor.matmul(
                            av[64:64 + VW, :], vsb[:, j * VW:(j + 1) * VW],
                            pt[:, k * CHUNK:(k + 1) * CHUNK],
                            start=(j == 0), stop=(j == jt - 1),
                            tile_position=(0, 64))

            pair_state = {}

            def normalize_a(p, tail=False):
                # drain the pair accumulator; move r rows to partition 0.
                # Mid-kernel 1/r runs on DVE (iterative divide, ~3.3us per
                # row, but DVE has slack); in the tail ScalarE is free, so
                # 1/r = exp(-ln r) there -- Log and Exp share a table set.
                av = av_box[0]
                m = wkp.tile([105, CHUNK], F32, tag='m')
                nc.vector.tensor_copy(m[:, :], av[0:105, :])
                r0 = wkp.tile([1, CHUNK], F32, tag='r0')
                r1 = wkp.tile([1, CHUNK], F32, tag='r1')
                nc.sync.dma_start(r0[0:1, :], m[DH:DH + 1, :])
                nc.sync.dma_start(r1[0:1, :], m[104:105, :])
                rec0 = wkp.tile([1, CHUNK], F32, tag='rec0')
                rec1 = wkp.tile([1, CHUNK], F32, tag='rec1')
                if tail:
                    ln0 = wkp.tile([1, CHUNK], F32, tag='ln0')
                    ln1 = wkp.tile([1, CHUNK], F32, tag='ln1')
                    nc.scalar.activation(ln0[0:1, :], r0[0:1, :], LOG)
                    nc.scalar.activation(ln1[0:1, :], r1[0:1, :], LOG)
                    nc.scalar.activation(rec0[0:1, :], ln0[0:1, :], EXP,
                                         scale=-1.0)
                    nc.scalar.activation(rec1[0:1, :], ln1[0:1, :], EXP,
                                         scale=-1.0)
                else:
                    nc.vector.reciprocal(rec0[0:1, :], r0[0:1, :])
                    nc.vector.reciprocal(rec1[0:1, :], r1[0:1, :])
                pair_state[p] = (m, rec0, rec1)

            def normalize_b_lo(p):
                # broadcast 1/r over partitions and scale into oT_lo
                m, rec0, rec1 = pair_state[p]
                c0s = slice(2 * p * CHUNK, (2 * p + 1) * CHUNK)
                rbc0 = ps_small.tile([DH, CHUNK], F32, tag='small')
                nc.tensor.matmul(rbc0[:, :], ones1[0:1, :], rec0[0:1, :],
                                 start=True, stop=True)
                rb0 = wkp.tile([DH, CHUNK], F32, tag='rb')
                nc.vector.tensor_copy(rb0[:, :], rbc0[:, :])
                nc.vector.tensor_tensor(
                    out=oT_lo[:, c0s], in0=m[0:DH, :], in1=rb0[:, :], op=MULT)

            def normalize_b_hi(p):
                m, rec0, rec1 = pair_state.pop(p)
                c1s = slice((2 * p + 1) * CHUNK, (2 * p + 2) * CHUNK)
                rbc1 = ps_small.tile([128, CHUNK], F32, tag='small')
                nc.tensor.matmul(rbc1[64:64 + DH, :], ones1[0:1, :],
                                 rec1[0:1, :], start=True, stop=True,
                                 tile_position=(0, 64))
                rb1 = wkp.tile([128, CHUNK], F32, tag='rb')
                nc.vector.tensor_copy(rb1[64:64 + DH, :], rbc1[64:64 + DH, :])
                nc.vector.tensor_tensor(
                    out=oT_hi[64:64 + DH, c1s], in0=m[64:64 + DH, :],
                    in1=rb1[64:64 + DH, :], op=MULT)

            def outproj_tile(st_i, hi):
                yp = ps_small.tile([128, D], F32, tag='small')
                if hi:
                    nc.tensor.matmul(yp[:, :],
                                     oT_hi[64:64 + DH,
                                           st_i * 128:(st_i + 1) * 128],
                                     woT2[64:64 + DH, :],
                                     start=True, stop=True)
                else:
                    nc.tensor.matmul(yp[:, :],
                                     oT_lo[:, st_i * 128:(st_i + 1) * 128],
                                     woT2[0:DH, :], start=True, stop=True)
                ysb = wkp.tile([128, D], F32, tag='ysb')
                nc.vector.tensor_copy(ysb[:, :], yp[:, :])
                nc.sync.dma_start(out_d[st_i * 128:(st_i + 1) * 128, :],
                                  ysb[:, :])

            def outproj_pair(p, t):
                # t in 0..7: lo tiles of chunk 2p first, then hi of 2p+1
                if t < tpc:
                    outproj_tile(2 * p * tpc + t, False)
                else:
                    outproj_tile((2 * p + 1) * tpc + (t - tpc), True)

            # ---- projections preamble (DMA-paced) ----
            # q/k drain from ps_st (B on DVE, A on ScalarE -- idle until the
            # ACT stream); v from ps_small. All three projections of chunk c
            # gate only on chunk c's xT slices.
            for c in range(n_chunks):
                proj_qk(c)
                proj(vT[0:DH, :], wv_sb, c, DH)

            # ---- main loop over i-chunks ----
            pts_prev = None
            for c in range(n_chunks):
                pts = []
                cs = slice(c * CHUNK, (c + 1) * CHUNK)
                for g in range(gpc):
                    st = ps_st.tile([128, GJ * CHUNK], F32, tag='st2')
                    j0, j1 = GJ * g, GJ * g + 1
                    nc.tensor.matmul(
                        st[:, 0:CHUNK],
                        qkB[0:DH, j0 * 128:(j0 + 1) * 128], qkA[0:DH, cs],
                        start=True, stop=True)
                    nc.tensor.matmul(
                        st[:, CHUNK:2 * CHUNK],
                        qkA[64:QKW, j1 * 128:(j1 + 1) * 128],
                        qkB[64:QKW, cs],
                        start=True, stop=True)
                    pt = ptp.tile([128, GJ * CHUNK], BF16, tag='pt')
                    nc.scalar.activation(pt[:, :], st[:, :], EXP, scale=SCALE)
                    pts.append(pt)

                    if c == 0:
                        # v' transposes at 2/slot -- with v-proj in the
                        # preamble, chunk 0 is a light 4-MM slot plus junk
                        transpose_v(2 * g)
                        transpose_v(2 * g + 1)
                        warm(2)
                    else:
                        av_mm(c - 1, pts_prev[g], g)
                        if c == n_chunks - 1 and g >= 1:
                            av_mm(c, pts[g - 1], g - 1)

                    # pair post-processing: normalize_a right after the
                    # pair's last AV stop (end of even chunks); the DVE
                    # reciprocal takes ~3.3us/row, so the broadcast and
                    # outproj are staggered deep into the following chunk
                    if c >= 2 and c % 2 == 0 and g == gpc - 1:
                        normalize_a((c - 2) // 2)
                    if c >= 3 and c % 2 == 1:
                        p = (c - 3) // 2
                        if g == 4:
                            normalize_b_lo(p)
                        elif g == 7:
                            normalize_b_hi(p)
                        elif 8 <= g:
                            outproj_pair(p, g - 8)

                    # junk keep-warm in slots that would otherwise idle
                    if c in (1, 2, 4, 6):
                        warm(1)
                    elif c in (3, 5) and g in (0, 1, 2, 3, 5, 6):
                        warm(1)
                pts_prev = pts

            # ---- tail: last AV group + last pair (1/r on idle ScalarE) ----
            av_mm(n_chunks - 1, pts_prev[gpc - 1], gpc - 1)
            p = n_chunks // 2 - 1
            normalize_a(p, tail=True)
            normalize_b_lo(p)
            normalize_b_hi(p)
            for t in range(2 * tpc):
                outproj_pair(p, t)

    if split:
        _split_sync_waits(nc)
    return nc


def kernel(x, Wq, Wk, Wv, Wo, bo):
    x = np.asarray(x, dtype=np.float32)
    Wq = np.asarray(Wq, dtype=np.float32)
    Wk = np.asarray(Wk, dtype=np.float32)
    Wv = np.asarray(Wv, dtype=np.float32)
    Wo = np.asarray(Wo, dtype=np.float32)
    bo = np.asarray(bo, dtype=np.float32)

    if 'nc' not in _COMPILED:
        _COMPILED['nc'] = _build()
    nc = _COMPILED['nc']

    bf = ml_dtypes.bfloat16
    xT = np.ascontiguousarray(x.reshape(S, D).T).astype(bf)
    in_maps = []
    for h in range(N_CORES):
        sl = slice(h * DH, (h + 1) * DH)
        wqT = Wq[sl, :].T  # [320, 40]
        wkT = Wk[sl, :].T
        wvT = Wv[sl, :].T
        # prepacked images: per 128-row K chunk, q at cols +0..39 and k at
        # +64..103 (wA), swapped for wB; v packed at 40-col stride
        wA = np.zeros((128, 3 * QKW), dtype=np.float32)
        wB = np.zeros((128, 3 * QKW), dtype=np.float32)
        wv = np.zeros((128, 3 * DH), dtype=np.float32)
        for ci, kk in enumerate((128, 128, 64)):
            o = (128, 128, 64)[0] * 0 + sum((128, 128, 64)[:ci])
            wA[0:kk, ci * QKW:ci * QKW + DH] = wqT[o:o + kk, :]
            wA[0:kk, ci * QKW + 64:ci * QKW + QKW] = wkT[o:o + kk, :]
            wB[0:kk, ci * QKW:ci * QKW + DH] = wkT[o:o + kk, :]
            wB[0:kk, ci * QKW + 64:ci * QKW + QKW] = wqT[o:o + kk, :]
            wv[0:kk, ci * DH:(ci + 1) * DH] = wvT[o:o + kk, :]
        woT = np.zeros((QKW, D), dtype=np.float32)
        woT[0:DH, :] = Wo[:, sl].T
        woT[64:64 + DH, :] = Wo[:, sl].T
        in_maps.append({
            'xT': xT,
            'wA': wA.astype(bf),
            'wB': wB.astype(bf),
            'wv': wv.astype(bf),
            'woT': woT.astype(bf),
        })

    trace = bool(os.environ.get('BASS_KERNEL_TRACE'))

    def _run():
        return bass_utils.run_bass_kernel_spmd(
            nc, in_maps, core_ids=list(range(N_CORES)), trace=trace,
            tmpdir=os.environ.get('BASS_KERNEL_TRACE_DIR') or None)

    try:
        res = _run()
    except Exception:
        # A previously crashed NEFF can leave the device unrecoverable; the
        # failed attempt clears it, so one retry is usually enough.
        res = _run()
    _COMPILED['last_res'] = res

    acc = res.results[0]['out'].astype(np.float32).copy()
    for h in range(1, N_CORES):
        acc += res.results[h]['out']
    acc += bo[None, :]
    return acc.reshape(1, S, D)


# revision 18
# speedup vs baseline: 1.1757x; 1.1757x over previous
"""CrossAttention (B=1, S=4096, H=8, DH=40) on 8 Trainium2 NeuronCores.

Sharding: tensor-parallel over the 8 heads — core h computes head h's full
attention plus its partial output projection; the host sums the 8 partials
and adds the bias.

v3 dataflow (vs v2: AV lagged a full chunk, HAM-warm junk matmuls, balanced
chunk-0, host-prepacked weight images, fast reciprocal, lean tail):
  preamble  wA/wB/wv/woT as single prepacked DMAs; xT in 1024-col slices
            across 3 issuing queues; dummy ACT preloads the exp table;
            qkA drains on ScalarE (idle until the ACT stream starts)
  qT/kT     packed [104, 4096] images (q@0/k@64 and k@0/q@64) so the two
            ST matmuls of a j-group run on disjoint PE row groups
  ST        [128j, 512i] pairs -> exp on ScalarE [128, 1024] (the wall:
            ~16.7M exps at 1 elem/cycle/lane = ~142us)
  AV        lag-16: av(c) consumes chunk c-1's exp tiles, so steady slots
            are uniform 2 ST + 2 AV and never wait on a fresh ACT; chunk 7
            carries av(6) and av(7) (lag-1) together
  warm      junk matmuls fill the ~260ns/slot PE idle of a ScalarE-paced
            steady state -- without them the HAM clock gate re-throttles
            the PE to 1.2 GHz (measured: half the kernel ran cold)
  norm      r rows 40/104 -> SBUF DMA to partition 0 -> recip_approx_fast
            -> K=1 ones-matmul broadcast -> mult into oT_lo/oT_hi; runs at
            the last slot of even chunks, right after the pair's AV stop
  Y         [128, 320] = oT.T @ Wo_h.T, spread 1 tile/slot over odd chunks
"""

import os

import ml_dtypes
import numpy as np

import concourse.bass as bass
import concourse.mybir as mybir
from concourse import bass_utils, masks
from concourse.tile import TileContext

S = 4096
D = 320
H = 8
DH = 40
N_CORES = 8
CHUNK = 512               # i-chunk width (one fp32 PSUM bank)
VW = 41                   # v' stationary width: 40 v cols, ones col 40
GJ = 2                    # j-tiles per exp group (2 PSUM banks)
SCALE = float(DH) ** -0.5
QKW = 104
JW = 384                  # junk keep-warm matmul moving width
BURST = int(os.environ.get('BASS_BURST', '40'))  # warm-up burst length

F32 = mybir.dt.float32
BF16 = mybir.dt.bfloat16
EXP = mybir.ActivationFunctionType.Exp
LOG = mybir.ActivationFunctionType.Ln
MULT = mybir.AluOpType.mult

KCH = (128, 128, 64)      # K chunks of D=320

_COMPILED = {}


def _split_sync_waits(nc, max_waits=1):
    """This walrus build rejects instructions with more than one sync wait.
    Spill the excess onto same-engine nops placed just before the
    instruction (engine streams execute in program order, so all waits are
    satisfied before the instruction issues)."""
    for f in nc.m.functions:
        for bb in f.blocks:
            out = []
            changed = False
            for inst in bb.instructions:
                si = inst.sync_info
                if si is not None and si.on_wait and len(si.on_wait) > max_waits:
                    waits = list(si.on_wait)
                    for i in range(max_waits, len(waits), max_waits):
                        nop = mybir.InstNoOp(
                            name=nc.get_next_instruction_name(),
                            engine=inst.engine,
                            bass_nofuse=True,
                            sync_info=mybir.SyncInfo(
                                on_wait=waits[i:i + max_waits], on_update=[]),
                        )
                        out.append(nop)
                    inst.sync_info = mybir.SyncInfo(
                        on_wait=waits[:max_waits],
                        on_update=list(si.on_update or []))
                    changed = True
                out.append(inst)
            if changed:
                bb.instructions = out


def _build(s=None, split=True):
    s = s or S
    n_chunks = s // CHUNK
    jt = s // 128
    gpc = jt // GJ            # exp groups per chunk
    tpc = CHUNK // 128        # s-tiles per chunk
    nc = bass.Bass('TRN2', target_bir_lowering=False, debug=False)

    xT_d = nc.dram_tensor('xT', [D, s], BF16, kind='ExternalInput').ap()
    wA_d = nc.dram_tensor('wA', [128, 3 * QKW], BF16, kind='ExternalInput').ap()
    wB_d = nc.dram_tensor('wB', [128, 3 * QKW], BF16, kind='ExternalInput').ap()
    wv_d = nc.dram_tensor('wv', [128, 3 * DH], BF16, kind='ExternalInput').ap()
    woT_d = nc.dram_tensor('woT', [QKW, D], BF16, kind='ExternalInput').ap()
    out_d = nc.dram_tensor('out', [s, D], F32, kind='ExternalOutput').ap()

    with TileContext(nc) as tc:
        with tc.tile_pool(name='const', bufs=1) as cpool, \
             tc.tile_pool(name='big', bufs=1) as big, \
             tc.tile_pool(name='pt', bufs=18) as ptp, \
             tc.tile_pool(name='work', bufs=3) as wkp, \
             tc.tile_pool(name='ps_st', bufs=2, space='PSUM') as ps_st, \
             tc.tile_pool(name='ps_small', bufs=2, space='PSUM') as ps_small, \
             tc.tile_pool(name='ps_av', bufs=1, space='PSUM') as ps_av, \
             tc.tile_pool(name='ps_warm', bufs=1, space='PSUM') as ps_warm:

            # ---- constants & inputs ----
            ident = cpool.tile([128, 128], F32, tag='ident')
            masks.make_identity(nc, ident[:, :])

            wA = cpool.tile([128, 3 * QKW], BF16, tag='wA')
            wB = cpool.tile([128, 3 * QKW], BF16, tag='wB')
            wv_sb = cpool.tile([128, 3 * DH], BF16, tag='wv')
            woT2 = cpool.tile([QKW, D], BF16, tag='woT')
            nc.sync.dma_start(wA[:, :], wA_d)
            nc.sync.dma_start(wB[:, :], wB_d)
            nc.scalar.dma_start(wv_sb[:, :], wv_d)
            nc.gpsimd.dma_start(woT2[:, :], woT_d)
            ones1 = cpool.tile([1, DH], F32, tag='ones1')
            nc.vector.memset(ones1[0:1, :], 1.0)

            # xT in 1024-col slices round-robined over 3 issuing queues so
            # the transfers overlap and proj(c) can start as slice c//2 lands
            xt0 = big.tile([128, s], BF16, tag='xt0')
            xt1 = big.tile([128, s], BF16, tag='xt1')
            xt2 = big.tile([64, s], BF16, tag='xt2')
            xts = (xt0, xt1, xt2)
            for sl in range(s // 1024):
                ss = slice(sl * 1024, (sl + 1) * 1024)
                nc.sync.dma_start(xt0[:, ss], xT_d[0:128, ss])
                nc.scalar.dma_start(xt1[:, ss], xT_d[128:256, ss])
                nc.gpsimd.dma_start(xt2[:, ss], xT_d[256:320, ss])

            qkA = big.tile([QKW, s], BF16, tag='qkA')
            qkB = big.tile([QKW, s], BF16, tag='qkB')
            vT = big.tile([VW, s], F32, tag='vT')
            vsb = big.tile([128, jt * VW], BF16, tag='vsb')
            oT_lo = big.tile([DH, s], BF16, tag='oT_lo')
            oT_hi = big.tile([QKW, s], BF16, tag='oT_hi')

            # base-32 memset (DVE base must be 32-aligned); proj then
            # overwrites rows 32..39 with v, leaving row 40 = 1.0
            nc.vector.memset(vT[32:VW, :], 1.0)

            # dummy ACT: forces the exp table-set load (~2.7us) to happen
            # during the DMA wait instead of delaying the first real exp
            sc = wkp.tile([1, 16], F32, tag='sc')
            nc.scalar.activation(sc[0:1, :], ident[0:1, 0:16], EXP)

            # warm-up burst: dense full-array junk matmuls during the DMA
            # wait. The PE clock release (1.2 -> 2.4 GHz) lags sustained
            # activity by ~100us across all measured variants; this front-
            # loads activity to pull the release earlier.
            jsrc = cpool.tile([128, CHUNK], BF16, tag='jsrc')
            nc.vector.memset(jsrc[:, :], 0.25)
            for _ in range(BURST):
                wj = ps_warm.tile([128, CHUNK], F32, tag='warm', name='wj')
                nc.tensor.matmul(wj[:, :], jsrc[:, 0:128], jsrc[:, :],
                                 start=True, stop=True)

            # ---- helpers ----
            def proj(dst, w_sb, c, ww, drain='vector'):
                ps = ps_small.tile([QKW, CHUNK], F32, tag='small')
                for ci, kk in enumerate(KCH):
                    nc.tensor.matmul(
                        ps[0:ww, :],
                        w_sb[0:kk, ci * ww:(ci + 1) * ww],
                        xts[ci][0:kk, c * CHUNK:(c + 1) * CHUNK],
                        start=(ci == 0), stop=(ci == 2))
                if drain == 'scalar':
                    nc.scalar.copy(dst[:, c * CHUNK:(c + 1) * CHUNK],
                                   ps[0:ww, :])
                else:
                    nc.vector.tensor_copy(dst[:, c * CHUNK:(c + 1) * CHUNK],
                                          ps[0:ww, :])

            def proj_qk(c):
                # q/k projections for chunk c into one 2-bank ps_st tile
                # (idle until the ST stream starts): B in the low bank, A in
                # the high bank. 2-chunk WAR distance keeps the PE free of
                # the drain-copy round-trip that stalled v3's chunk 0.
                ps = ps_st.tile([128, GJ * CHUNK], F32, tag='st2')
                cs = slice(c * CHUNK, (c + 1) * CHUNK)
                for ci, kk in enumerate(KCH):
                    nc.tensor.matmul(
                        ps[0:QKW, 0:CHUNK],
                        wB[0:kk, ci * QKW:(ci + 1) * QKW],
                        xts[ci][0:kk, cs],
                        start=(ci == 0), stop=(ci == 2))
                for ci, kk in enumerate(KCH):
                    nc.tensor.matmul(
                        ps[0:QKW, CHUNK:2 * CHUNK],
                        wA[0:kk, ci * QKW:(ci + 1) * QKW],
                        xts[ci][0:kk, cs],
                        start=(ci == 0), stop=(ci == 2))
                nc.vector.tensor_copy(qkB[:, cs], ps[0:QKW, 0:CHUNK])
                nc.scalar.copy(qkA[:, cs], ps[0:QKW, CHUNK:2 * CHUNK])

            def transpose_v(j):
                tp = ps_small.tile([128, VW], F32, tag='small')
                nc.tensor.transpose(tp[:, 0:VW], vT[:, j * 128:(j + 1) * 128],
                                    ident[0:VW, 0:VW])
                nc.vector.tensor_copy(vsb[:, j * VW:(j + 1) * VW],
                                      tp[:, 0:VW])

            def warm(n=1):
                # keep-warm matmuls: a ScalarE-paced steady state leaves the
                # PE idle ~260ns/slot, which trips the HAM activity monitor
                # and halves the PE clock. Junk matmuls fill the gap.
                w = ps_warm.tile([128, CHUNK], F32, tag='warm', name='warm')
                for _ in range(n):
                    nc.tensor.matmul(w[0:QKW, 0:JW], wA[:, 0:QKW],
                                     xt0[:, 0:JW], start=True, stop=True)

            av_box = [None]

            def av_mm(c_src, pt, g):
                # two j-tile matmuls of group g, consuming chunk c_src's exp
                # tiles: even chunk at PE cols 0-40, odd at 64-104. The last
                # pair accumulates in the ps_warm bank (free in chunk 7, no
                # junk there) so it never WARs on the previous pair's drain.
                lo = (c_src % 2 == 0)
                for k in range(GJ):
                    j = GJ * g + k
                    if j == 0 and lo:
                        pool = ps_warm if c_src >= 6 else ps_av
                        av_box[0] = pool.tile([128, CHUNK], F32,
                                              tag='warm' if c_src >= 6
                                              else 'av',
                                              name='av')
                    av = av_box[0]
                    if lo:
                        nc.tensor.matmul(
                            av[0:VW, :], vsb[:, j * VW:(j + 1) * VW],
                            pt[:, k * CHUNK:(k + 1) * CHUNK],
                            start=(j == 0), stop=(j == jt - 1))
                    else:
                        nc.tensor.matmul(
                            av[64:64 + VW, :], vsb[:, j * VW:(j + 1) * VW],
                            pt[:, k * CHUNK:(k + 1) * CHUNK],
                            start=(j == 0), stop=(j == jt - 1),
                            tile_position=(0, 64))

            pair_state = {}

            def normalize_a(p, tail=False):
                # drain the pair accumulator; move r rows to partition 0.
                # Mid-kernel 1/r runs on DVE (iterative divide, ~3.3us per
                # row, but DVE has slack); in the tail ScalarE is free, so
                # 1/r = exp(-ln r) there -- Log and Exp share a table set.
                av = av_box[0]
                m = wkp.tile([105, CHUNK], F32, tag='m')
                nc.vector.tensor_copy(m[:, :], av[0:105, :])
                r0 = wkp.tile([1, CHUNK], F32, tag='r0')
                r1 = wkp.tile([1, CHUNK], F32, tag='r1')
                nc.sync.dma_start(r0[0:1, :], m[DH:DH + 1, :])
                nc.sync.dma_start(r1[0:1, :], m[104:105, :])
                rec0 = wkp.tile([1, CHUNK], F32, tag='rec0')
                rec1 = wkp.tile([1, CHUNK], F32, tag='rec1')
                nc.vector.reciprocal(rec0[0:1, :], r0[0:1, :])
                if not tail:
                    nc.vector.reciprocal(rec1[0:1, :], r1[0:1, :])
                else:
                    # tail: defer rec1 so the lo-half normalize + outproj
                    # overlap the second 3.3us reciprocal
                    pair_state['tail_r1'] = (rec1, r1)
                pair_state[p] = (m, rec0, rec1)

            def tail_recip1():
                rec1, r1 = pair_state.pop('tail_r1')
                nc.vector.reciprocal(rec1[0:1, :], r1[0:1, :])

            def normalize_b_lo(p):
                # broadcast 1/r over partitions and scale into oT_lo
                m, rec0, rec1 = pair_state[p]
                c0s = slice(2 * p * CHUNK, (2 * p + 1) * CHUNK)
                rbc0 = ps_small.tile([DH, CHUNK], F32, tag='small')
                nc.tensor.matmul(rbc0[:, :], ones1[0:1, :], rec0[0:1, :],
                                 start=True, stop=True)
                rb0 = wkp.tile([DH, CHUNK], F32, tag='rb')
                nc.vector.tensor_copy(rb0[:, :], rbc0[:, :])
                nc.vector.tensor_tensor(
                    out=oT_lo[:, c0s], in0=m[0:DH, :], in1=rb0[:, :], op=MULT)

            def normalize_b_hi(p):
                m, rec0, rec1 = pair_state.pop(p)
                c1s = slice((2 * p + 1) * CHUNK, (2 * p + 2) * CHUNK)
                rbc1 = ps_small.tile([128, CHUNK], F32, tag='small')
                nc.tensor.matmul(rbc1[64:64 + DH, :], ones1[0:1, :],
                                 rec1[0:1, :], start=True, stop=True,
                                 tile_position=(0, 64))
                rb1 = wkp.tile([128, CHUNK], F32, tag='rb')
                nc.vector.tensor_copy(rb1[64:64 + DH, :], rbc1[64:64 + DH, :])
                nc.vector.tensor_tensor(
                    out=oT_hi[64:64 + DH, c1s], in0=m[64:64 + DH, :],
                    in1=rb1[64:64 + DH, :], op=MULT)

            def outproj_tile(st_i, hi):
                yp = ps_small.tile([128, D], F32, tag='small')
                if hi:
                    nc.tensor.matmul(yp[:, :],
                                     oT_hi[64:64 + DH,
                                           st_i * 128:(st_i + 1) * 128],
                                     woT2[64:64 + DH, :],
                                     start=True, stop=True)
                else:
                    nc.tensor.matmul(yp[:, :],
                                     oT_lo[:, st_i * 128:(st_i + 1) * 128],
                                     woT2[0:DH, :], start=True, stop=True)
                ysb = wkp.tile([128, D], F32, tag='ysb')
                nc.vector.tensor_copy(ysb[:, :], yp[:, :])
                nc.sync.dma_start(out_d[st_i * 128:(st_i + 1) * 128, :],
                                  ysb[:, :])

            def outproj_pair(p, t):
                # t in 0..7: lo tiles of chunk 2p first, then hi of 2p+1
                if t < tpc:
                    outproj_tile(2 * p * tpc + t, False)
                else:
                    outproj_tile((2 * p + 1) * tpc + (t - tpc), True)

            # ---- projections preamble (DMA-paced) ----
            # q/k drain from ps_st (B on DVE, A on ScalarE -- idle until the
            # ACT stream); v from ps_small. All three projections of chunk c
            # gate only on chunk c's xT slices.
            for c in range(n_chunks):
                proj_qk(c)
                proj(vT[0:DH, :], wv_sb, c, DH)

            # ---- main loop over i-chunks ----
            pts_prev = None
            for c in range(n_chunks):
                pts = []
                cs = slice(c * CHUNK, (c + 1) * CHUNK)
                for g in range(gpc):
                    st = ps_st.tile([128, GJ * CHUNK], F32, tag='st2')
                    j0, j1 = GJ * g, GJ * g + 1
                    nc.tensor.matmul(
                        st[:, 0:CHUNK],
                        qkB[0:DH, j0 * 128:(j0 + 1) * 128], qkA[0:DH, cs],
                        start=True, stop=True)
                    nc.tensor.matmul(
                        st[:, CHUNK:2 * CHUNK],
                        qkA[64:QKW, j1 * 128:(j1 + 1) * 128],
                        qkB[64:QKW, cs],
                        start=True, stop=True)
                    pt = ptp.tile([128, GJ * CHUNK], BF16, tag='pt')
                    nc.scalar.activation(pt[:, :], st[:, :], EXP, scale=SCALE)
                    pts.append(pt)

                    if c == 0:
                        # v' transposes at 2/slot -- with v-proj in the
                        # preamble, chunk 0 is a light 4-MM slot plus junk
                        transpose_v(2 * g)
                        transpose_v(2 * g + 1)
                        warm(2)
                    else:
                        av_mm(c - 1, pts_prev[g], g)
                        if c == n_chunks - 1 and g >= 1:
                            av_mm(c, pts[g - 1], g - 1)

                    # pair post-processing: normalize_a right after the
                    # pair's last AV stop (end of even chunks); the DVE
                    # reciprocal takes ~3.3us/row, so the broadcast and
                    # outproj are staggered deep into the following chunk
                    if c >= 2 and c % 2 == 0 and g == gpc - 1:
                        normalize_a((c - 2) // 2)
                    if c >= 3 and c % 2 == 1:
                        p = (c - 3) // 2
                        if g == 4:
                            normalize_b_lo(p)
                        elif g == 7:
                            normalize_b_hi(p)
                        elif 8 <= g:
                            outproj_pair(p, g - 8)

                    # junk keep-warm in slots that would otherwise idle
                    if c in (1, 2, 4, 6):
                        warm(1)
                    elif c in (3, 5) and g in (0, 1, 2, 3, 5, 6):
                        warm(1)
                pts_prev = pts

            # ---- tail: last AV group + last pair; lo-half outproj overlaps
            # the hi-half reciprocal ----
            av_mm(n_chunks - 1, pts_prev[gpc - 1], gpc - 1)
            p = n_chunks // 2 - 1
            normalize_a(p, tail=True)
            normalize_b_lo(p)
            tail_recip1()
            for t in range(tpc):
                outproj_pair(p, t)
            normalize_b_hi(p)
            for t in range(tpc, 2 * tpc):
                outproj_pair(p, t)

    if split:
        _split_sync_waits(nc)
    return nc


def kernel(x, Wq, Wk, Wv, Wo, bo):
    x = np.asarray(x, dtype=np.float32)
    Wq = np.asarray(Wq, dtype=np.float32)
    Wk = np.asarray(Wk, dtype=np.float32)
    Wv = np.asarray(Wv, dtype=np.float32)
    Wo = np.asarray(Wo, dtype=np.float32)
    bo = np.asarray(bo, dtype=np.float32)

    if 'nc' not in _COMPILED:
        _COMPILED['nc'] = _build()
    nc = _COMPILED['nc']

    bf = ml_dtypes.bfloat16
    xT = np.ascontiguousarray(x.reshape(S, D).T).astype(bf)
    in_maps = []
    for h in range(N_CORES):
        sl = slice(h * DH, (h + 1) * DH)
        wqT = Wq[sl, :].T  # [320, 40]
        wkT = Wk[sl, :].T
        wvT = Wv[sl, :].T
        # prepacked images: per 128-row K chunk, q at cols +0..39 and k at
        # +64..103 (wA), swapped for wB; v packed at 40-col stride
        wA = np.zeros((128, 3 * QKW), dtype=np.float32)
        wB = np.zeros((128, 3 * QKW), dtype=np.float32)
        wv = np.zeros((128, 3 * DH), dtype=np.float32)
        for ci, kk in enumerate((128, 128, 64)):
            o = (128, 128, 64)[0] * 0 + sum((128, 128, 64)[:ci])
            wA[0:kk, ci * QKW:ci * QKW + DH] = wqT[o:o + kk, :]
            wA[0:kk, ci * QKW + 64:ci * QKW + QKW] = wkT[o:o + kk, :]
            wB[0:kk, ci * QKW:ci * QKW + DH] = wkT[o:o + kk, :]
            wB[0:kk, ci * QKW + 64:ci * QKW + QKW] = wqT[o:o + kk, :]
            wv[0:kk, ci * DH:(ci + 1) * DH] = wvT[o:o + kk, :]
        woT = np.zeros((QKW, D), dtype=np.float32)
        woT[0:DH, :] = Wo[:, sl].T
        woT[64:64 + DH, :] = Wo[:, sl].T
        in_maps.append({
            'xT': xT,
            'wA': wA.astype(bf),
            'wB': wB.astype(bf),
            'wv': wv.astype(bf),
            'woT': woT.astype(bf),
        })

    trace = bool(os.environ.get('BASS_KERNEL_TRACE'))

    def _run():
        return bass_utils.run_bass_kernel_spmd(
            nc, in_maps, core_ids=list(range(N_CORES)), trace=trace,
            tmpdir=os.environ.get('BASS_KERNEL_TRACE_DIR') or None)

    try:
        res = _run()
    except Exception:
        # A previously crashed NEFF can leave the device unrecoverable; the
        # failed attempt clears it, so one retry is usually enough.
        res = _run()
    _COMPILED['last_res'] = res

    acc = res.results[0]['out'].astype(np.float32).copy()
    for h in range(1, N_CORES):
        acc += res.results[h]['out']
    acc += bo[None, :]
    return acc.reshape(1, S, D)
